# revision 1
# baseline (speedup 1.0000x reference)
"""Trainium2 Bass kernel for nn_CAGetBoard (neural CA step).

Takes FULL inputs, shards batch across 8 NeuronCores (pure data parallel),
runs a Bass/Tile kernel per core, gathers the FULL output.

Per-core pipeline (B/8 images each), all matmuls bf16:
  - conv1 (Sobel folded into a 16->128 3x3 conv) = 2 accumulating matmuls
    (K=48 left-tap + K=96 center/right) over a 6-copy row/col-shifted
    stacked x (bf16, cast once to a DRAM scratch); 258-stride rows with
    zeroed pad columns give W-edge SAME padding via rhs free offsets.
  - relu+bias drains PSUM->SBUF bf16 (ACT/DVE 3:1), paired 2-chunk drains.
  - mm2 (128->16) col-tiled x4 with duplicated weights (M=32 at col bases
    0/32/64/96) -> packed [128,512] PSUM -> single tanh(+bias) drain; the
    d channel-3 rows stream back to row-layout tiles for the alive mask.
  - alive masks in row-layout; 3x3 binary dilation via banded bf16 matmuls
    + horizontal adds; u/alive replicated to the packed channel layout by
    SWDGE broadcast DMAs.
  - finishing: boards = clip_or_id((x + d*u) * alive) via full-width TTs
    + one fused min/max tensor_scalar.
"""

import numpy as np

import concourse.bass as bass
import concourse.bacc as bacc
import concourse.tile as tile
import concourse.mybir as mybir
from concourse.bass_utils import run_bass_kernel_spmd

dt = mybir.dt
F32 = dt.float32
F32R = dt.float32r
BF16 = dt.bfloat16
AF = mybir.ActivationFunctionType
OP = mybir.AluOpType

N_CORES = 8
C = 16
H = 256
W = 256
TR = 32                    # rows per compute block
WS = W + 2                 # padded row stride
N_BLK = H // TR
N_CHUNK = TR // 2          # 512-px chunks per block
N_GRP = N_CHUNK // 4       # mm2 groups per block
PX_IMG = H * W
EPS = 0.5
ALIVE_T = 0.1


def _build_consts(w1, b1, w2, b2):
    w1 = np.asarray(w1, np.float32)
    w2 = np.asarray(w2, np.float32)
    sob = np.array([[-1., 0., 1.], [-2., 0., 2.], [-1., 0., 1.]], np.float32)
    W1x, W1gx, W1gy = w1[:, 0:16], w1[:, 16:32], w1[:, 32:48]
    k1f = (W1gx[:, :, None, None] * sob[None, None, :, :]
           + W1gy[:, :, None, None] * sob.T[None, None, :, :])
    k1f[:, :, 1, 1] += W1x
    lhs = np.transpose(k1f, (3, 2, 1, 0)).reshape(3, 48, 128)
    lhsA = lhs[0].copy()
    lhsB = np.concatenate([lhs[1], lhs[2]], axis=0)

    w2dup = np.zeros((128, 32), np.float32)
    w2dup[:, 0:16] = w2.T
    w2dup[:, 16:32] = w2.T

    b2dup = np.zeros((128, 1), np.float32)
    for i in range(4):
        for d in range(2):
            s = 32 * i + 16 * d
            b2dup[s:s + 16, 0] = b2

    ones4 = np.zeros((4, 128), np.float32)
    for k in range(4):
        ones4[k, 32 * k:32 * k + 32] = 1.0

    bandB = np.zeros((128, 128), np.float32)
    for k in range(128):
        bandB[k, max(0, k - 1):k + 2] = 1.0
    bandClo = np.zeros((128, 128), np.float32)
    bandClo[0, 127] = 1.0
    bandChi = np.zeros((128, 128), np.float32)
    bandChi[127, 0] = 1.0
    clo1 = np.zeros((1, 128), np.float32)
    clo1[0, 127] = 1.0

    return dict(
        lhsA=lhsA, lhsB=lhsB,
        w2dup=w2dup, b1c=np.asarray(b1, np.float32).reshape(128, 1),
        b2dup=b2dup, ones4=ones4,
        bandB=bandB, bandClo=bandClo, bandChi=bandChi, clo1=clo1,
    )


CONST_SPECS = dict(
    lhsA=([48, 128], BF16), lhsB=([96, 128], BF16),
    w2dup=([128, 32], BF16), b1c=([128, 1], F32), b2dup=([128, 1], F32),
    ones4=([4, 128], BF16),
    bandB=([128, 128], BF16), bandClo=([128, 128], BF16),
    bandChi=([128, 128], BF16), clo1=([1, 128], BF16),
)


def build_program(n_img, reps=1):
    nc = bacc.Bacc("TRN2", target_bir_lowering=False)

    x_d = nc.dram_tensor("x", [n_img, C, H, W], F32, kind="ExternalInput")
    rand_d = nc.dram_tensor("rand", [n_img, H, W], F32, kind="ExternalInput")
    cst_d = {k: nc.dram_tensor(k, sh, d, kind="ExternalInput")
             for k, (sh, d) in CONST_SPECS.items()}
    out_d = nc.dram_tensor("out", [n_img, C, H, W], F32, kind="ExternalOutput")
    alive_d = nc.dram_tensor("alivescr", [n_img, PX_IMG], BF16, kind="Internal")
    xbf_d = nc.dram_tensor("xbfscr", [n_img, C, H, W], BF16, kind="Internal")

    xf = x_d.ap().rearrange("b c h w -> b c (h w)")
    outf = out_d.ap().rearrange("b c h w -> b c (h w)")
    randf = rand_d.ap().rearrange("b h w -> b (h w)")

    with tile.TileContext(nc) as tc:
        xbf = xbf_d.ap().rearrange("b c h w -> b c (h w)")
        _emit(nc, tc, n_img, xf, randf, cst_d, outf, alive_d.ap(),
              xbf, reps)
    nc.compile()
    return nc


def _emit(nc, tc, n_img, xf, randf, cst_d, outf, alivef, xbf, reps=1):
    from contextlib import ExitStack
    ctx = ExitStack()

    def pool(name, bufs, **kw):
        return ctx.enter_context(tc.tile_pool(name=name, bufs=bufs, **kw))

    consts = pool("consts", 1)
    stackp = pool("stack", 1)
    hgrp_p = pool("hgrp", 4)
    dgrp_p = pool("dgrp", 6)
    fin_p = pool("fin", 2)
    fs_p = pool("fs", 4)
    row_p = pool("rows", 4)
    rowsm_p = pool("rowsm", 2)
    misc_p = pool("misc", 1)
    conv_ps = pool("convps", 2, space="PSUM")
    mask_ps = pool("maskps", 1, space="PSUM")
    mm2_ps = pool("mm2ps", 2, space="PSUM")

    cst = {}
    for k, (sh, d) in CONST_SPECS.items():
        t = consts.tile(sh, d, tag=k, name=k)
        nc.sync.dma_start(t[:], cst_d[k].ap())
        cst[k] = t

    zeros = misc_p.tile([128, 1024], F32, tag="zeros", name="zeros")
    nc.vector.memset(zeros[:], 0.0)

    stacks = []
    for s in range(3):
        st = stackp.tile([96, TR * WS], BF16, tag=f"stack{s}", name=f"stack{s}")
        st3 = st.rearrange("p (r j) -> p r j", j=WS)
        nc.vector.memset(st3[:, :, 0:1], 0.0)
        nc.vector.memset(st3[:, :, W + 1:W + 2], 0.0)
        stacks.append(st3)

    sdil = []
    for s in range(4):
        t = misc_p.tile([128, WS], F32, tag=f"sdil{s}", name=f"sdil{s}")
        nc.vector.memset(t[:, 0:1], 0.0)
        nc.vector.memset(t[:, W + 1:W + 2], 0.0)
        sdil.append(t)

    def dilate_half(half, b_main, extra_lhs, extra_rhs, out_t, sgrp=0):
        """out = dilate3x3(binary) for one 128-row half.
        vertical: bandB.T @ b_main + extra_lhs.T @ extra_rhs, then horizontal
        adds on a 258-padded drain tile, then > 0.5."""
        vs = mask_ps.tile([128, W], F32, tag="mask", name="vs")
        nc.tensor.matmul(vs[:], cst["bandB"][:], b_main[:],
                         start=True, stop=(extra_lhs is None))
        if extra_lhs is not None:
            nc.tensor.matmul(vs[:], extra_lhs, extra_rhs,
                             start=False, stop=True)
        s = sdil[2 * sgrp + half]
        nc.scalar.activation(s[:, 1:W + 1], vs[:], AF.Copy)
        t = rowsm_p.tile([128, W], F32, tag="dil_t", name="dil_t")
        nc.vector.tensor_add(t[:], s[:, 0:W], s[:, 2:W + 2])
        nc.vector.tensor_add(t[:], t[:], s[:, 1:W + 1])
        nc.vector.tensor_single_scalar(out_t[:], t[:], 0.5, OP.is_gt)

    for b in range(n_img):
        nc.gpsimd.dma_start(xbf[b], xf[b])

    for b in [i for _ in range(reps) for i in range(n_img)]:
        # ---------------- row-layout pre-pass ----------------
        x3row, randrow, bpre, prealive = [], [], [], []
        for half in range(2):
            xt = row_p.tile([128, W], F32, tag="x3row", name="x3row")
            nc.sync.dma_start(
                xt[:], xf[b, 3, half * 128 * W:(half + 1) * 128 * W]
                .rearrange("(p w) -> p w", w=W))
            x3row.append(xt)
            rt = row_p.tile([128, W], F32, tag="randrow", name="randrow")
            nc.sync.dma_start(
                rt[:], randf[b, half * 128 * W:(half + 1) * 128 * W]
                .rearrange("(p w) -> p w", w=W))
            randrow.append(rt)
            bt = row_p.tile([128, W], BF16, tag="bpre", name="bpre")
            nc.vector.tensor_single_scalar(bt[:], xt[:], ALIVE_T, OP.is_gt)
            bpre.append(bt)
            prealive.append(row_p.tile([128, W], BF16, tag="prealive", name="prealive"))
        dilate_half(0, bpre[0], cst["bandClo"][:], bpre[1][:], prealive[0])
        dilate_half(1, bpre[1], cst["bandChi"][:], bpre[0][:], prealive[1])

        dkeep = {}
        d3row = []
        for half in range(2):
            t = row_p.tile([128, W], BF16, tag="d3row", name="d3row")
            d3row.append(t)

        def compute_block(blk):
            r0 = blk * TR
            dgb = dgrp_p.tile([128, 2048], BF16, tag="d", name="d")
            dkeep[blk] = dgb
            st3 = stacks[blk % 3]
            if blk == 0:
                nc.vector.memset(st3[0:32, 0:1, :], 0.0)
                nc.vector.memset(st3[32:64, 0:1, :], 0.0)
            if blk == N_BLK - 1:
                nc.vector.memset(st3[32:64, TR - 1:TR, :], 0.0)
                nc.vector.memset(st3[64:96, TR - 1:TR, :], 0.0)
            nc.vector.memset(st3[32:64, :, W:W + 1], 0.0)
            nc.vector.memset(st3[64:96, :, W:W + 1], 0.0)
            for di in range(3):
                rr_lo = max(0, 1 - di - r0)
                rr_hi = min(TR, H - r0 - di + 1)
                srcA = xbf[b, :, (r0 + rr_lo + di - 1) * W:
                           (r0 + rr_hi + di - 1) * W].rearrange(
                               "c (r w) -> c r w", w=W)
                englist = (nc.sync, nc.scalar)
                englist[di % 2].dma_start(
                    st3[16 * di:16 * di + 16, rr_lo:rr_hi, 1:W + 1], srcA)
                srcB = xbf[b, :, (r0 + rr_lo + di - 1) * W:
                           (r0 + rr_hi + di - 1) * W].rearrange(
                               "c (r w) -> c r w", w=W)[:, :, 1:W]
                englist[(di + 1) % 2].dma_start(
                    st3[48 + 16 * di:64 + 16 * di, rr_lo:rr_hi, 1:W], srcB)


            for g in range(N_GRP):
                hg = hgrp_p.tile([128, 2048], BF16, tag="hgrp", name="hgrp")
                for ip in range(2):
                    acc = conv_ps.tile([128, 1024], F32, tag="conv",
                                       name="conv")
                    for i in (2 * ip, 2 * ip + 1):
                        chk = 4 * i + g
                        asl = acc[:, 512 * (i - 2 * ip):512 * (i - 2 * ip + 1)]
                        nc.tensor.matmul(
                            asl, cst["lhsA"][:],
                            st3[0:48, 2 * chk:2 * chk + 2, 0:W],
                            start=True, stop=False)
                        nc.tensor.matmul(
                            asl, cst["lhsB"][:],
                            st3[0:96, 2 * chk:2 * chk + 2, 1:W + 1],
                            start=False, stop=True)
                    hsl = hg[:, 1024 * ip:1024 * (ip + 1)]
                    if (2 * g + ip) % 4 != 3:
                        nc.scalar.activation(hsl, acc[:], AF.Relu,
                                             bias=cst["b1c"][:, 0:1])
                    else:
                        nc.vector.scalar_tensor_tensor(
                            hsl, acc[:], cst["b1c"][:, 0:1], zeros[:],
                            op0=OP.add, op1=OP.max)
                mm = mm2_ps.tile([128, 512], F32, tag="mm2", name="mm2")
                for i in range(4):
                    nc.tensor.matmul(
                        mm[32 * i:32 * i + 32, :],
                        cst["w2dup"][:],
                        hg[:, 512 * i:512 * (i + 1)],
                        start=True, stop=True,
                        tile_position=(0, 32 * i))
                nc.scalar.activation(dgb[:, 512 * g:512 * (g + 1)], mm[:],
                                     AF.Tanh, bias=cst["b2dup"][:, 0:1])
            j0 = blk * N_CHUNK
            half = blk // 4
            for i in range(4):
                r = (2 * (j0 + 4 * i)) % 128
                nc.scalar.dma_start(d3row[half][r:r + 8, :],
                                    dgb[32 * i + 3:32 * i + 4, :])

        def post_binary(rows_ap_rand, rows_ap_x3, d3_ap, out_t):
            """out = (x3 + d3*(rand<eps)) > 0.1  on row-layout tiles."""
            m = rowsm_p.tile(list(out_t.shape), F32, tag="postm", name="postm")
            nc.vector.scalar_tensor_tensor(
                m[:], rows_ap_rand, EPS, d3_ap,
                op0=OP.is_lt, op1=OP.mult)
            nc.vector.tensor_add(m[:], m[:], rows_ap_x3)
            nc.vector.tensor_single_scalar(out_t[:], m[:], ALIVE_T, OP.is_gt)

        def alive_store(half, ar):
            nc.sync.dma_start(
                alivef[b, half * 128 * W:(half + 1) * 128 * W]
                .rearrange("(p w) -> p w", w=W), ar[:])

        def finish_block(blk):
            j0 = blk * N_CHUNK
            px0 = j0 * 512
            # x in dup layout: 4 DMAs of [32, 2048] (dup-pair folded)
            xd = fin_p.tile([128, 2048], F32, tag="xdup", name="xdup")
            for i in range(4):
                nc.sync.dma_start(
                    xd[32 * i:32 * i + 32, :],
                    xf[b, :, px0 + 2048 * i:px0 + 2048 * (i + 1)]
                    .unsqueeze(0).broadcast_to([2, C, 2048]))
            # u4 = (rand < 0.5) exact f32 cmp -> bf16 [4, 2048]
            r4 = fin_p.tile([4, 2048], F32, tag="r4", name="r4", bufs=1)
            nc.sync.dma_start(
                r4[:], randf[b, px0:px0 + 8192]
                .rearrange("(i n) -> i n", n=2048))
            u4 = fin_p.tile([4, 2048], BF16, tag="u4", name="u4")
            nc.vector.tensor_single_scalar(u4[:], r4[:], EPS, OP.is_lt)
            dgb = dkeep.pop(blk)
            t = fs_p.tile([128, 2048], F32, tag="fs", name="t")
            t2 = fs_p.tile([128, 2048], F32, tag="fs", name="t2")
            t3 = fs_p.tile([128, 2048], F32, tag="fs", name="t3")
            u16 = fin_p.tile([128, 2048], BF16, tag="u16", name="u16")
            a16 = fin_p.tile([128, 2048], BF16, tag="a16", name="a16")
            for i in range(4):
                nc.gpsimd.dma_start(
                    u16[32 * i:32 * i + 32, :],
                    u4[i:i + 1, :].unsqueeze(1)
                    .broadcast_to([1, 32, 2048]))
                nc.gpsimd.dma_start(
                    a16[32 * i:32 * i + 32, :],
                    alivef[b, px0 + 2048 * i:px0 + 2048 * (i + 1)]
                    .unsqueeze(0).broadcast_to([32, 2048]))
            nc.vector.tensor_mul(t[:], dgb[:], u16[:])
            nc.vector.tensor_add(t2[:], t[:], xd[:])
            nc.vector.tensor_mul(t3[:], t2[:], a16[:])
            t4 = fs_p.tile([128, 2048], F32, tag="fs", name="t4")
            nc.vector.tensor_scalar(t4[:], t3[:], 1.0, 0.0,
                                    op0=OP.min, op1=OP.max)
            for i in range(4):
                eng = nc.sync if i % 2 == 0 else nc.scalar
                eng.dma_start(
                    outf[b, 0:3, px0 + 2048 * i:px0 + 2048 * (i + 1)],
                    t4[32 * i:32 * i + 3, :])
                eng.dma_start(
                    outf[b, 3:16, px0 + 2048 * i:px0 + 2048 * (i + 1)],
                    t3[32 * i + 3:32 * i + 16, :])

        # ---------------- pipeline ----------------
        for blk in range(5):
            compute_block(blk)

        # post binary for half 0 + row 128 (first row of block 4)
        bpost0 = rowsm_p.tile([128, W], BF16, tag="bpost0", name="bpost0")
        post_binary(randrow[0][:], x3row[0][:], d3row[0][:], bpost0)
        bp128 = rowsm_p.tile([1, W], BF16, tag="bp128", name="bp128")
        post_binary(randrow[1][0:1, :], x3row[1][0:1, :], d3row[1][0:1, :],
                    bp128)

        postal0 = rowsm_p.tile([128, W], BF16, tag="postal0", name="postal0")
        dilate_half(0, bpost0, cst["clo1"][:], bp128[:], postal0, sgrp=1)
        ar0 = rowsm_p.tile([128, W], BF16, tag="ar0", name="ar0")
        nc.vector.tensor_mul(ar0[:], prealive[0][:], postal0[:])
        alive_store(0, ar0)

        compute_block(5)
        finish_block(0)
        compute_block(6)
        finish_block(1)
        compute_block(7)
        finish_block(2)
        finish_block(3)

        bpost1 = rowsm_p.tile([128, W], BF16, tag="bpost1", name="bpost1")
        post_binary(randrow[1][:], x3row[1][:], d3row[1][:], bpost1)
        postal1 = rowsm_p.tile([128, W], BF16, tag="postal1", name="postal1")
        dilate_half(1, bpost1, cst["bandChi"][:], bpost0[:], postal1, sgrp=1)
        ar1 = rowsm_p.tile([128, W], BF16, tag="ar1", name="ar1")
        nc.vector.tensor_mul(ar1[:], prealive[1][:], postal1[:])
        alive_store(1, ar1)

        for blk in range(4, 8):
            finish_block(blk)

    ctx.close()


# ---------------------------------------------------------------------------

_NC_CACHE = {}


def _get_nc(n_img, reps=1):
    key = (n_img, reps)
    if key not in _NC_CACHE:
        _NC_CACHE[key] = build_program(n_img, reps)
    return _NC_CACHE[key]


def kernel(x, w1, b1, w2, b2, rand_mask):
    x = np.ascontiguousarray(np.asarray(x, np.float32))
    rand_mask = np.ascontiguousarray(np.asarray(rand_mask, np.float32))
    B = x.shape[0]
    n_img = B // N_CORES
    consts = _build_consts(w1, b1, w2, b2)
    cast = {k: np.ascontiguousarray(v.astype(mybir.dt.np(CONST_SPECS[k][1])))
            for k, v in consts.items()}

    nc = _get_nc(n_img)
    in_maps = []
    for k in range(N_CORES):
        sl = slice(k * n_img, (k + 1) * n_img)
        in_maps.append(dict(x=x[sl], rand=rand_mask[sl, 0], **cast))
    res = run_bass_kernel_spmd(nc, in_maps, core_ids=list(range(N_CORES)))
    out = np.concatenate([res.results[k]["out"] for k in range(N_CORES)],
                         axis=0)
    return out.astype(np.float32)



# revision 30
# speedup vs baseline: 1.6837x; 1.6837x over previous
"""Trainium2 Bass kernel for nn_CAGetBoard (neural CA step).

Takes FULL inputs, shards batch across 8 NeuronCores (pure data parallel),
runs a Bass/Tile kernel per core, gathers the FULL output.

Per-core pipeline (B/8 images each):
  - conv1 (Sobel folded into a 16->128 3x3 conv) = 2 accumulating matmuls
    (K=48 left-tap + K=96 center/right) over a 6-copy row/col-shifted
    stacked x (bf16); 258-stride rows with zeroed pad columns give W-edge
    SAME padding via rhs free offsets.
  - relu+bias drains PSUM->SBUF bf16 (split ACT/DVE).
  - mm2 (128->16) col-tiled x4 with duplicated weights -> packed [128,512]
    PSUM -> single tanh(+bias) drain into dgb.
  - d is written to a ch-major DRAM scratch (dscr[c, px]) per block, then
    read back per half-image in "ch-row" layout [128p=px-rows, 16c, 256]
    where all per-pixel masks (u = rand<eps, alive) broadcast across the
    ch axis with stride-0 APs -- no mask replication DMAs.
  - alive masks as in baseline: 3x3 binary dilation via banded bf16
    matmuls + horizontal adds on [128,256] row tiles; kept in SBUF.
  - finishing per half in bf16: m=d*u; n=m+x16; o=n*alive; clip ch<3
    (contiguous columns in ch-row layout); one SWDGE store casting
    bf16->f32.
"""

import numpy as np

import concourse.bass as bass
import concourse.bacc as bacc
import concourse.tile as tile
import concourse.mybir as mybir
from concourse.bass_utils import run_bass_kernel_spmd

dt = mybir.dt
F32 = dt.float32
BF16 = dt.bfloat16
AF = mybir.ActivationFunctionType
OP = mybir.AluOpType

N_CORES = 8
C = 16
H = 256
W = 256
TR = 32                    # rows per compute block
WS = W + 2                 # padded row stride
N_BLK = H // TR
N_CHUNK = TR // 2          # 512-px chunks per block
N_GRP = N_CHUNK // 4       # mm2 groups per block
PX_IMG = H * W
HPX = PX_IMG // 2          # pixels per half-image
EPS = 0.5
ALIVE_T = 0.1
DRAIN_ACT_OF_16 = 9        # of every 16 relu drains, this many go to ACT
CONV_FP8 = True            # conv1 via fp8e4m3 DoubleRow on (x - 0.5)
FP8 = dt.float8e4


def _build_consts(w1, b1, w2, b2):
    w1 = np.asarray(w1, np.float32)
    w2 = np.asarray(w2, np.float32)
    sob = np.array([[-1., 0., 1.], [-2., 0., 2.], [-1., 0., 1.]], np.float32)
    W1x, W1gx, W1gy = w1[:, 0:16], w1[:, 16:32], w1[:, 32:48]
    k1f = (W1gx[:, :, None, None] * sob[None, None, :, :]
           + W1gy[:, :, None, None] * sob.T[None, None, :, :])
    k1f[:, :, 1, 1] += W1x
    lhs = np.transpose(k1f, (3, 2, 1, 0)).reshape(3, 48, 128)
    lhsA = lhs[0].copy()
    lhsB = np.concatenate([lhs[1], lhs[2]], axis=0)

    # fp8 DoubleRow pair weights: [48, 2, 128] (slab0, slab1)
    lhsA8 = np.stack([lhs[0], lhs[1]], axis=1)
    lhsB8 = np.stack([np.zeros_like(lhs[2]), lhs[2]], axis=1)
    # bias correction for the x-0.5 shift: conv(w, x) =
    # conv(w, x-0.5) + 0.5*sum_taps(w)
    ksum = k1f.sum(axis=(1, 2, 3)) * 0.5
    b1fp8 = (np.asarray(b1, np.float32) + ksum).reshape(128, 1)

    w2dup = np.zeros((128, 32), np.float32)
    w2dup[:, 0:16] = w2.T
    w2dup[:, 16:32] = w2.T

    b2dup = np.zeros((128, 1), np.float32)
    for i in range(4):
        for d in range(2):
            s = 32 * i + 16 * d
            b2dup[s:s + 16, 0] = b2

    bandB = np.zeros((128, 128), np.float32)
    for k in range(128):
        bandB[k, max(0, k - 1):k + 2] = 1.0
    bandClo = np.zeros((128, 128), np.float32)
    bandClo[0, 127] = 1.0
    bandChi = np.zeros((128, 128), np.float32)
    bandChi[127, 0] = 1.0
    clo1 = np.zeros((1, 128), np.float32)
    clo1[0, 127] = 1.0

    b1c = (b1fp8 if CONV_FP8
           else np.asarray(b1, np.float32).reshape(128, 1))
    return dict(
        lhsA=lhsA, lhsB=lhsB, lhsA8=lhsA8.reshape(48, 256),
        lhsB8=lhsB8.reshape(48, 256),
        w2dup=w2dup, b1c=b1c,
        b2dup=b2dup,
        bandB=bandB, bandClo=bandClo, bandChi=bandChi, clo1=clo1,
    )


CONST_SPECS = dict(
    lhsA=([48, 128], BF16), lhsB=([96, 128], BF16),
    lhsA8=([48, 256], FP8), lhsB8=([48, 256], FP8),
    w2dup=([128, 32], BF16), b1c=([128, 1], F32), b2dup=([128, 1], F32),
    bandB=([128, 128], BF16), bandClo=([128, 128], BF16),
    bandChi=([128, 128], BF16), clo1=([1, 128], BF16),
)


def build_program(n_img, reps=1):
    nc = bacc.Bacc("TRN2", target_bir_lowering=False)

    x_d = nc.dram_tensor("x", [n_img, C, H, W], F32, kind="ExternalInput")
    rand_d = nc.dram_tensor("rand", [n_img, H, W], F32, kind="ExternalInput")
    cst_d = {k: nc.dram_tensor(k, sh, d, kind="ExternalInput")
             for k, (sh, d) in CONST_SPECS.items()}
    out_d = nc.dram_tensor("out", [n_img, C, H, W], F32, kind="ExternalOutput")
    # d scratch in drow order: [img, half, p(=row in half), c, w]
    dscr_d = nc.dram_tensor("dscr", [n_img, 2, 128, C, W], BF16,
                            kind="Internal")

    xf = x_d.ap().rearrange("b c h w -> b c (h w)")
    outf = out_d.ap().rearrange("b c h w -> b c (h w)")
    randf = rand_d.ap().rearrange("b h w -> b (h w)")

    if CONV_FP8:
        xs_d = nc.dram_tensor("xsh8", [n_img, C, H, W], FP8,
                              kind="ExternalInput")
        xsrc = xs_d.ap().rearrange("b c h w -> b c (h w)")
    else:
        xbf_d = nc.dram_tensor("xbfscr", [n_img, C, H, W], BF16,
                               kind="Internal")
        xsrc = xbf_d.ap().rearrange("b c h w -> b c (h w)")

    with tile.TileContext(nc) as tc:
        _emit(nc, tc, n_img, xf, randf, cst_d, outf, xsrc, dscr_d.ap(), reps)
    nc.compile()
    return nc


def _emit(nc, tc, n_img, xf, randf, cst_d, outf, xsrc, dscrf, reps=1):
    from contextlib import ExitStack
    ctx = ExitStack()

    def pool(name, bufs, **kw):
        return ctx.enter_context(tc.tile_pool(name=name, bufs=bufs, **kw))

    consts = pool("consts", 1)
    stackp = pool("stack", 1)
    hgrp_p = pool("hgrp", 4)
    dgrp_p = pool("dgrp", 3)
    row_p = pool("rows", 4)
    rowsm_p = pool("rowsm", 2)
    fin_p = pool("fin", 2)
    misc_p = pool("misc", 1)
    conv_ps = pool("convps", 2, space="PSUM")
    mask_ps = pool("maskps", 1, space="PSUM")
    mm2_ps = pool("mm2ps", 2, space="PSUM")

    cst = {}
    for k, (sh, d) in CONST_SPECS.items():
        t = consts.tile(sh, d, tag=k, name=k)
        nc.sync.dma_start(t[:], cst_d[k].ap())
        cst[k] = t

    zeros = misc_p.tile([128, 1024], F32, tag="zeros", name="zeros")
    nc.vector.memset(zeros[:], 0.0)

    # stacks.  A-layout: col k = x col k-1 (stored at cols 1..256, col 0
    # zero).  B-layout: col k = x col k (stored at cols 0..255, col 256
    # zero).  Full 256-elem rows on all DMAs.
    # bf16 path: [96, r, j] with A on partitions 0:48, B on 48:96.
    # fp8 path: [48, slab, r, j] with A = slab 0, B = slab 1 (DoubleRow).
    stacks = []
    for s in range(3):
        if CONV_FP8:
            st = stackp.tile([48, 2 * TR * WS], FP8, tag=f"stack{s}",
                             name=f"stack{s}")
            st4 = st.rearrange("p (s r j) -> p s r j", s=2, j=WS)
            nc.vector.memset(st4[0:48, 0, :, 0:1], -0.5)
            nc.vector.memset(st4[0:48, 1, :, W:W + 1], -0.5)
            stacks.append(st4)
        else:
            st = stackp.tile([96, TR * WS], BF16, tag=f"stack{s}",
                             name=f"stack{s}")
            st3 = st.rearrange("p (r j) -> p r j", j=WS)
            nc.vector.memset(st3[0:96, :, 0:1], 0.0)
            nc.vector.memset(st3[0:96, :, W:W + 1], 0.0)
            stacks.append(st3)

    sdil = []
    for s in range(4):
        t = misc_p.tile([128, WS], F32, tag=f"sdil{s}", name=f"sdil{s}")
        nc.vector.memset(t[:, 0:1], 0.0)
        nc.vector.memset(t[:, W + 1:W + 2], 0.0)
        sdil.append(t)

    def dilate_half(half, b_main, extra_lhs, extra_rhs, out_t, sgrp=0):
        """out = dilate3x3(binary) for one 128-row half."""
        vs = mask_ps.tile([128, W], F32, tag="mask", name="vs")
        nc.tensor.matmul(vs[:], cst["bandB"][:], b_main[:],
                         start=True, stop=(extra_lhs is None))
        if extra_lhs is not None:
            nc.tensor.matmul(vs[:], extra_lhs, extra_rhs,
                             start=False, stop=True)
        s = sdil[2 * sgrp + half]
        nc.scalar.activation(s[:, 1:W + 1], vs[:], AF.Copy)
        t = rowsm_p.tile([128, W], F32, tag="dil_t", name="dil_t")
        nc.vector.tensor_add(t[:], s[:, 0:W], s[:, 2:W + 2])
        nc.vector.tensor_add(t[:], t[:], s[:, 1:W + 1])
        nc.vector.tensor_single_scalar(out_t[:], t[:], 0.5, OP.is_gt)

    if not CONV_FP8:
        for b in range(n_img):
            for q in range(4):
                nc.gpsimd.dma_start(xsrc[b, :, q * (PX_IMG // 4):
                                    (q + 1) * (PX_IMG // 4)],
                                    xf[b, :, q * (PX_IMG // 4):
                                    (q + 1) * (PX_IMG // 4)])

    lhsA8v = cst["lhsA8"][:].rearrange("p (s m) -> p s m", s=2)
    lhsB8v = cst["lhsB8"][:].rearrange("p (s m) -> p s m", s=2)
    DR = mybir.MatmulPerfMode.DoubleRow

    for b in [i for _ in range(reps) for i in range(n_img)]:
        # ---------------- row-layout pre-pass ----------------
        x3row, randrow, urow, bpre, prealive = [], [], [], [], []
        for half in range(2):
            xt = row_p.tile([128, W], F32, tag="x3row", name="x3row")
            nc.sync.dma_start(
                xt[:], xf[b, 3, half * HPX:(half + 1) * HPX]
                .rearrange("(p w) -> p w", w=W))
            x3row.append(xt)
            rt = row_p.tile([128, W], F32, tag="randrow", name="randrow")
            nc.sync.dma_start(
                rt[:], randf[b, half * HPX:(half + 1) * HPX]
                .rearrange("(p w) -> p w", w=W))
            randrow.append(rt)
            ut = row_p.tile([128, W], BF16, tag="urow", name="urow")
            nc.vector.tensor_single_scalar(ut[:], rt[:], EPS, OP.is_lt)
            urow.append(ut)
            bt = row_p.tile([128, W], BF16, tag="bpre", name="bpre")
            nc.vector.tensor_single_scalar(bt[:], xt[:], ALIVE_T, OP.is_gt)
            bpre.append(bt)
            prealive.append(row_p.tile([128, W], BF16, tag="prealive",
                                       name="prealive"))
        dilate_half(0, bpre[0], cst["bandClo"][:], bpre[1][:], prealive[0])
        dilate_half(1, bpre[1], cst["bandChi"][:], bpre[0][:], prealive[1])

        # x in ch-row layout per half: [128, 16, 256] bf16 (SWDGE cast),
        # split into 32-partition pieces to keep DMA queue granularity fine
        xrow = []
        for half in range(2):
            t = fin_p.tile([128, C * W], BF16, tag="xrow", name="xrow")
            for q in range(4):
                nc.gpsimd.dma_start(
                    t[32 * q:32 * (q + 1), :]
                    .rearrange("p (c n) -> p c n", n=W),
                    xf[b, :, half * HPX + 32 * q * W:
                       half * HPX + 32 * (q + 1) * W]
                    .rearrange("c (p n) -> p c n", n=W))
            xrow.append(t)

        def load_drow(half, name):
            dr = fin_p.tile([128, C * W], BF16, tag="drow", name=name)
            for hq in range(2):
                nc.sync.dma_start(
                    dr[64 * hq:64 * (hq + 1), :]
                    .rearrange("p (c w) -> p c w", w=W),
                    dscrf[b, half, 64 * hq:64 * (hq + 1)])
            return dr

        def compute_block(blk):
            r0 = blk * TR
            st3 = stacks[blk % 3]
            englist = (nc.sync, nc.scalar)
            if CONV_FP8:
                if blk == 0:
                    nc.vector.memset(st3[0:16, :, 0:1, :], -0.5)
                if blk == N_BLK - 1:
                    nc.vector.memset(st3[32:48, :, TR - 1:TR, :], -0.5)
                for di in range(3):
                    rr_lo = max(0, 1 - di - r0)
                    rr_hi = min(TR, H - r0 - di + 1)
                    src = xsrc[b, :, (r0 + rr_lo + di - 1) * W:
                               (r0 + rr_hi + di - 1) * W].rearrange(
                                   "c (r w) -> c r w", w=W)
                    # A-slab: cols 1..256 <- x cols 0..255
                    englist[di % 2].dma_start(
                        st3[16 * di:16 * di + 16, 0, rr_lo:rr_hi, 1:W + 1],
                        src)
                    # B-slab: cols 0..255 <- x cols 0..255
                    englist[(di + 1) % 2].dma_start(
                        st3[16 * di:16 * di + 16, 1, rr_lo:rr_hi, 0:W], src)
            else:
                if blk == 0:
                    nc.vector.memset(st3[0:32, 0:1, :], 0.0)
                    nc.vector.memset(st3[32:64, 0:1, :], 0.0)
                if blk == N_BLK - 1:
                    nc.vector.memset(st3[32:64, TR - 1:TR, :], 0.0)
                    nc.vector.memset(st3[64:96, TR - 1:TR, :], 0.0)
                for di in range(3):
                    rr_lo = max(0, 1 - di - r0)
                    rr_hi = min(TR, H - r0 - di + 1)
                    src = xsrc[b, :, (r0 + rr_lo + di - 1) * W:
                               (r0 + rr_hi + di - 1) * W].rearrange(
                                   "c (r w) -> c r w", w=W)
                    # A-group: cols 1..257 <- x cols 0..255
                    englist[di % 2].dma_start(
                        st3[16 * di:16 * di + 16, rr_lo:rr_hi, 1:W + 1], src)
                    # B-group: cols 0..255 <- x cols 0..255 (full row)
                    englist[(di + 1) % 2].dma_start(
                        st3[48 + 16 * di:64 + 16 * di, rr_lo:rr_hi, 0:W], src)

            dgb = dgrp_p.tile([128, 2048], BF16, tag="d", name="d")
            for g in range(N_GRP):
                hg = hgrp_p.tile([128, 2048], BF16, tag="hgrp", name="hgrp")
                for ip in range(2):
                    acc = conv_ps.tile([128, 1024], F32, tag="conv",
                                       name="conv")
                    for i in (2 * ip, 2 * ip + 1):
                        chk = 4 * i + g
                        base = 512 * (i - 2 * ip)
                        if CONV_FP8:
                            for r in range(2):
                                aslr = acc[:, base + 256 * r:
                                           base + 256 * (r + 1)]
                                rr = 2 * chk + r
                                nc.tensor.matmul(
                                    aslr, lhsA8v, st3[0:48, :, rr, 0:W],
                                    start=True, stop=False, perf_mode=DR)
                                nc.tensor.matmul(
                                    aslr, lhsB8v, st3[0:48, :, rr, 1:W + 1],
                                    start=False, stop=True, perf_mode=DR)
                        else:
                            asl = acc[:, base:base + 512]
                            nc.tensor.matmul(
                                asl, cst["lhsA"][:],
                                st3[0:48, 2 * chk:2 * chk + 2, 0:W],
                                start=True, stop=False)
                            nc.tensor.matmul(
                                asl, cst["lhsB"][:],
                                st3[0:96, 2 * chk:2 * chk + 2, 1:W + 1],
                                start=False, stop=True)
                    hsl = hg[:, 1024 * ip:1024 * (ip + 1)]
                    if (blk * 8 + 2 * g + ip) % 16 < DRAIN_ACT_OF_16:
                        nc.scalar.activation(hsl, acc[:], AF.Relu,
                                             bias=cst["b1c"][:, 0:1])
                    else:
                        nc.vector.scalar_tensor_tensor(
                            hsl, acc[:], cst["b1c"][:, 0:1], zeros[:],
                            op0=OP.add, op1=OP.max)
                mm = mm2_ps.tile([128, 512], F32, tag="mm2", name="mm2")
                for i in range(4):
                    nc.tensor.matmul(
                        mm[32 * i:32 * i + 32, :],
                        cst["w2dup"][:],
                        hg[:, 512 * i:512 * (i + 1)],
                        start=True, stop=True,
                        tile_position=(0, 32 * i))
                nc.scalar.activation(dgb[:, 512 * g:512 * (g + 1)], mm[:],
                                     AF.Tanh, bias=cst["b2dup"][:, 0:1])
            # d -> DRAM scratch in drow order; image row of chunk 4i+g =
            # 32qb + 8i + 2g + r.  Per-i pieces keep SBUF dim0 = partition.
            half, qb = divmod(blk, 4)
            for i in range(4):
                eng = nc.sync if i % 2 == 0 else nc.scalar
                eng.dma_start(
                    dscrf[b, half, 32 * qb + 8 * i:32 * qb + 8 * (i + 1)]
                    .rearrange("p c w -> c p w"),
                    dgb[32 * i:32 * i + 16, :]
                    .rearrange("c (g r w) -> c (g r) w", r=2, w=W))

        def post_binary(rows_ap_rand, rows_ap_x3, d3_ap, out_t):
            """out = (x3 + d3*(rand<eps)) > 0.1 on row-layout tiles."""
            m = rowsm_p.tile(list(out_t.shape), F32, tag="postm", name="postm")
            nc.vector.scalar_tensor_tensor(
                m[:], rows_ap_rand, EPS, d3_ap,
                op0=OP.is_lt, op1=OP.mult)
            nc.vector.tensor_add(m[:], m[:], rows_ap_x3)
            nc.vector.tensor_single_scalar(out_t[:], m[:], ALIVE_T, OP.is_gt)

        def finish_half(half, ar, dr):
            """o = clip_ch<3(ar * (xrow + d*u)); cast-stores to out."""
            dr3 = dr[:].rearrange("p (c n) -> p c n", n=W)
            ub = urow[half][:].unsqueeze(1).broadcast_to([128, C, W])
            ab = ar[:].unsqueeze(1).broadcast_to([128, C, W])
            m = fin_p.tile([128, C * W], BF16, tag="finm", name="finm")
            m3 = m[:].rearrange("p (c n) -> p c n", n=W)
            nc.vector.tensor_mul(m3, dr3, ub)
            n_ = fin_p.tile([128, C * W], BF16, tag="finn", name="finn")
            n3 = n_[:].rearrange("p (c n) -> p c n", n=W)
            nc.vector.tensor_add(n3, m3,
                                 xrow[half][:].rearrange("p (c n) -> p c n",
                                                         n=W))
            o = fin_p.tile([128, C * W], BF16, tag="finm", name="fino")
            o3 = o[:].rearrange("p (c n) -> p c n", n=W)
            nc.vector.tensor_mul(o3, n3, ab)
            # clip channels 0..2 = contiguous cols 0..767, in place
            nc.vector.tensor_scalar(o[:, 0:3 * W], o[:, 0:3 * W], 1.0, 0.0,
                                    op0=OP.min, op1=OP.max)
            for q in range(4):
                nc.gpsimd.dma_start(
                    outf[b, :, half * HPX + 32 * q * W:
                         half * HPX + 32 * (q + 1) * W]
                    .rearrange("c (p n) -> p c n", n=W),
                    o3[32 * q:32 * (q + 1)])

        # ---------------- pipeline ----------------
        for blk in range(4):
            compute_block(blk)
        dr0 = load_drow(0, "drow0")
        compute_block(4)

        # half-0 post mask: d3 rows 0..127 from drow0 + halo row 128
        d3halo = rowsm_p.tile([1, W], BF16, tag="d3halo", name="d3halo")
        nc.sync.dma_start(d3halo[:], dscrf[b, 1, 0:1, 3, :])

        bpost0 = rowsm_p.tile([128, W], BF16, tag="bpost0", name="bpost0")
        post_binary(randrow[0][:], x3row[0][:],
                    dr0[:].rearrange("p (c n) -> p c n", n=W)[:, 3, :],
                    bpost0)
        bp128 = rowsm_p.tile([1, W], BF16, tag="bp128", name="bp128")
        post_binary(randrow[1][0:1, :], x3row[1][0:1, :], d3halo[:], bp128)

        postal0 = rowsm_p.tile([128, W], BF16, tag="postal0", name="postal0")
        dilate_half(0, bpost0, cst["clo1"][:], bp128[:], postal0, sgrp=1)
        ar0 = rowsm_p.tile([128, W], BF16, tag="ar0", name="ar0")
        nc.vector.tensor_mul(ar0[:], prealive[0][:], postal0[:])

        compute_block(5)
        finish_half(0, ar0, dr0)
        compute_block(6)
        compute_block(7)

        # half-1 post mask + finish
        dr1 = load_drow(1, "drow1")
        bpost1 = rowsm_p.tile([128, W], BF16, tag="bpost1", name="bpost1")
        post_binary(randrow[1][:], x3row[1][:],
                    dr1[:].rearrange("p (c n) -> p c n", n=W)[:, 3, :],
                    bpost1)
        postal1 = rowsm_p.tile([128, W], BF16, tag="postal1", name="postal1")
        dilate_half(1, bpost1, cst["bandChi"][:], bpost0[:], postal1, sgrp=1)
        ar1 = rowsm_p.tile([128, W], BF16, tag="ar1", name="ar1")
        nc.vector.tensor_mul(ar1[:], prealive[1][:], postal1[:])
        finish_half(1, ar1, dr1)

    ctx.close()


# ---------------------------------------------------------------------------

_NC_CACHE = {}


def _get_nc(n_img, reps=1):
    key = (n_img, reps)
    if key not in _NC_CACHE:
        _NC_CACHE[key] = build_program(n_img, reps)
    return _NC_CACHE[key]


def kernel(x, w1, b1, w2, b2, rand_mask):
    x = np.ascontiguousarray(np.asarray(x, np.float32))
    rand_mask = np.ascontiguousarray(np.asarray(rand_mask, np.float32))
    B = x.shape[0]
    n_img = B // N_CORES
    consts = _build_consts(w1, b1, w2, b2)
    cast = {k: np.ascontiguousarray(v.astype(mybir.dt.np(CONST_SPECS[k][1])))
            for k, v in consts.items()}

    nc = _get_nc(n_img)
    xsh8 = None
    if CONV_FP8:
        xsh8 = np.ascontiguousarray((x - 0.5).astype(mybir.dt.np(FP8)))
    in_maps = []
    for k in range(N_CORES):
        sl = slice(k * n_img, (k + 1) * n_img)
        m = dict(x=x[sl], rand=rand_mask[sl, 0], **cast)
        if CONV_FP8:
            m["xsh8"] = xsh8[sl]
        in_maps.append(m)
    res = run_bass_kernel_spmd(nc, in_maps, core_ids=list(range(N_CORES)))
    out = np.concatenate([res.results[k]["out"] for k in range(N_CORES)],
                         axis=0)
    return out.astype(np.float32)


# revision 43
# speedup vs baseline: 1.9787x; 1.1752x over previous
"""Trainium2 Bass kernel for nn_CAGetBoard (neural CA step).

Takes FULL inputs, shards batch across 8 NeuronCores (pure data parallel),
runs a Bass/Tile kernel per core, gathers the FULL output.

Per-core pipeline (B/8 images each, 4 blocks of 64 rows):
  - conv1 (Sobel folded into a 16->128 3x3 conv) runs in fp8e4m3
    DoubleRow: the host supplies xsh8 = (x - 0.5) in e4m3 (the shift
    halves quantization error; 0.5*sum(w) is folded into b1).  The
    6-copy stack is [48, 2(slab), r, 258] with A/B column alignments as
    slabs, so 2 DoubleRow matmuls (K=2x48) per 256-px row cover all 9
    taps at half cost; pads are -0.5 (the shifted-domain zero).
  - relu+bias drains PSUM->SBUF bf16, split ACT/DVE 11:5; mm2 (128->16,
    bf16, col-tiled x4 with duplicated weights) is software-pipelined
    one group behind conv; tanh drains into dgb.
  - d goes to a DRAM scratch in row-major (drow) order via per-i
    partition-strided stores; d3 mask rows are extracted straight from
    dgb.  Finishing reads d back per half-image in "ch-row" layout
    [128p=rows, 16c, 256] where u = rand<eps and the alive mask
    broadcast across the ch axis with stride-0 APs (no replication
    DMAs); all-bf16 m=d*u; n=m+x; o=n*alive; clip ch<3 = contiguous
    columns; SWDGE stores cast bf16->f32.
  - alive masks: 3x3 binary dilation via banded bf16 matmuls +
    horizontal adds on [128,256] row tiles, all kept in SBUF.
  - cross-image software pipelining: each image's half-1 masks+finish
    are deferred into the next image's block stream; DMA issue is
    spread across the SP (stack feeds), ACT (d3/dscr) and Pool
    (drow/x/out, casts) queues to avoid sequencer head-of-line
    blocking.
"""

import numpy as np

import concourse.bass as bass
import concourse.bacc as bacc
import concourse.tile as tile
import concourse.mybir as mybir
from concourse.bass_utils import run_bass_kernel_spmd

dt = mybir.dt
F32 = dt.float32
BF16 = dt.bfloat16
AF = mybir.ActivationFunctionType
OP = mybir.AluOpType

N_CORES = 8
C = 16
H = 256
W = 256
TR = 64                    # rows per compute block
WS = W + 2                 # padded row stride
N_BLK = H // TR
N_CHUNK = TR // 2          # 512-px chunks per block
N_GRP = N_CHUNK // 4       # mm2 groups per block
PX_IMG = H * W
HPX = PX_IMG // 2          # pixels per half-image
EPS = 0.5
ALIVE_T = 0.1
DRAIN_ACT_OF_16 = 9        # of every 16 relu drains, this many go to ACT
CONV_FP8 = True            # conv1 via fp8e4m3 DoubleRow on (x - 0.5)
FP8 = dt.float8e4


def _build_consts(w1, b1, w2, b2):
    w1 = np.asarray(w1, np.float32)
    w2 = np.asarray(w2, np.float32)
    sob = np.array([[-1., 0., 1.], [-2., 0., 2.], [-1., 0., 1.]], np.float32)
    W1x, W1gx, W1gy = w1[:, 0:16], w1[:, 16:32], w1[:, 32:48]
    k1f = (W1gx[:, :, None, None] * sob[None, None, :, :]
           + W1gy[:, :, None, None] * sob.T[None, None, :, :])
    k1f[:, :, 1, 1] += W1x
    lhs = np.transpose(k1f, (3, 2, 1, 0)).reshape(3, 48, 128)
    lhsA = lhs[0].copy()
    lhsB = np.concatenate([lhs[1], lhs[2]], axis=0)

    # fp8 DoubleRow pair weights: [48, 2, 128] (slab0, slab1)
    lhsA8 = np.stack([lhs[0], lhs[1]], axis=1)
    lhsB8 = np.stack([np.zeros_like(lhs[2]), lhs[2]], axis=1)
    # bias correction for the x-0.5 shift: conv(w, x) =
    # conv(w, x-0.5) + 0.5*sum_taps(w)
    ksum = k1f.sum(axis=(1, 2, 3)) * 0.5
    b1fp8 = (np.asarray(b1, np.float32) + ksum).reshape(128, 1)

    w2dup = np.zeros((128, 32), np.float32)
    w2dup[:, 0:16] = w2.T
    w2dup[:, 16:32] = w2.T

    b2dup = np.zeros((128, 1), np.float32)
    for i in range(4):
        for d in range(2):
            s = 32 * i + 16 * d
            b2dup[s:s + 16, 0] = b2

    bandB = np.zeros((128, 128), np.float32)
    for k in range(128):
        bandB[k, max(0, k - 1):k + 2] = 1.0
    bandClo = np.zeros((128, 128), np.float32)
    bandClo[0, 127] = 1.0
    bandChi = np.zeros((128, 128), np.float32)
    bandChi[127, 0] = 1.0
    clo1 = np.zeros((1, 128), np.float32)
    clo1[0, 127] = 1.0

    b1c = (b1fp8 if CONV_FP8
           else np.asarray(b1, np.float32).reshape(128, 1))
    return dict(
        lhsA=lhsA, lhsB=lhsB, lhsA8=lhsA8.reshape(48, 256),
        lhsB8=lhsB8.reshape(48, 256),
        w2dup=w2dup, b1c=b1c,
        b2dup=b2dup,
        bandB=bandB, bandClo=bandClo, bandChi=bandChi, clo1=clo1,
    )


CONST_SPECS = dict(
    lhsA=([48, 128], BF16), lhsB=([96, 128], BF16),
    lhsA8=([48, 256], FP8), lhsB8=([48, 256], FP8),
    w2dup=([128, 32], BF16), b1c=([128, 1], F32), b2dup=([128, 1], F32),
    bandB=([128, 128], BF16), bandClo=([128, 128], BF16),
    bandChi=([128, 128], BF16), clo1=([1, 128], BF16),
)


def build_program(n_img, reps=1):
    nc = bacc.Bacc("TRN2", target_bir_lowering=False)

    x_d = nc.dram_tensor("x", [n_img, C, H, W], F32, kind="ExternalInput")
    rand_d = nc.dram_tensor("rand", [n_img, H, W], F32, kind="ExternalInput")
    cst_d = {k: nc.dram_tensor(k, sh, d, kind="ExternalInput")
             for k, (sh, d) in CONST_SPECS.items()}
    out_d = nc.dram_tensor("out", [n_img, C, H, W], F32, kind="ExternalOutput")
    # d scratch in drow order: [img, half, p(=row in half), c, w]
    dscr_d = nc.dram_tensor("dscr", [n_img, 2, 128, C, W], BF16,
                            kind="Internal")

    xf = x_d.ap().rearrange("b c h w -> b c (h w)")
    outf = out_d.ap().rearrange("b c h w -> b c (h w)")
    randf = rand_d.ap().rearrange("b h w -> b (h w)")

    if CONV_FP8:
        xs_d = nc.dram_tensor("xsh8", [n_img, C, H, W], FP8,
                              kind="ExternalInput")
        xsrc = xs_d.ap().rearrange("b c h w -> b c (h w)")
    else:
        xbf_d = nc.dram_tensor("xbfscr", [n_img, C, H, W], BF16,
                               kind="Internal")
        xsrc = xbf_d.ap().rearrange("b c h w -> b c (h w)")

    with tile.TileContext(nc) as tc:
        _emit(nc, tc, n_img, xf, randf, cst_d, outf, xsrc, dscr_d.ap(), reps)
    nc.compile()
    return nc


def _emit(nc, tc, n_img, xf, randf, cst_d, outf, xsrc, dscrf, reps=1):
    from contextlib import ExitStack
    ctx = ExitStack()

    def pool(name, bufs, **kw):
        return ctx.enter_context(tc.tile_pool(name=name, bufs=bufs, **kw))

    consts = pool("consts", 1)
    stackp = pool("stack", 1)
    hgrp_p = pool("hgrp", 4)
    dgrp_p = pool("dgrp", 3)
    row_p = pool("rows", 4)
    rowsm_p = pool("rowsm", 2)
    d3_p = pool("d3", 4)
    fin_p = pool("fin", 2)
    misc_p = pool("misc", 1)
    conv_ps = pool("convps", 2, space="PSUM")
    mask_ps = pool("maskps", 1, space="PSUM")
    mm2_ps = pool("mm2ps", 2, space="PSUM")

    cst = {}
    for k, (sh, d) in CONST_SPECS.items():
        t = consts.tile(sh, d, tag=k, name=k)
        nc.sync.dma_start(t[:], cst_d[k].ap())
        cst[k] = t

    zeros = misc_p.tile([128, 1024], F32, tag="zeros", name="zeros")
    nc.vector.memset(zeros[:], 0.0)

    # stacks.  A-layout: col k = x col k-1 (stored at cols 1..256, col 0
    # zero).  B-layout: col k = x col k (stored at cols 0..255, col 256
    # zero).  Full 256-elem rows on all DMAs.
    # bf16 path: [96, r, j] with A on partitions 0:48, B on 48:96.
    # fp8 path: [48, slab, r, j] with A = slab 0, B = slab 1 (DoubleRow).
    stacks = []
    for s in range(2):
        if CONV_FP8:
            st = stackp.tile([48, 2 * TR * WS], FP8, tag=f"stack{s}",
                             name=f"stack{s}")
            st4 = st.rearrange("p (s r j) -> p s r j", s=2, j=WS)
            nc.vector.memset(st4[0:48, 0, :, 0:1], -0.5)
            nc.vector.memset(st4[0:48, 1, :, W:W + 1], -0.5)
            stacks.append(st4)
        else:
            st = stackp.tile([96, TR * WS], BF16, tag=f"stack{s}",
                             name=f"stack{s}")
            st3 = st.rearrange("p (r j) -> p r j", j=WS)
            nc.vector.memset(st3[0:96, :, 0:1], 0.0)
            nc.vector.memset(st3[0:96, :, W:W + 1], 0.0)
            stacks.append(st3)

    sdil = []
    for s in range(4):
        t = misc_p.tile([128, WS], F32, tag=f"sdil{s}", name=f"sdil{s}")
        nc.vector.memset(t[:, 0:1], 0.0)
        nc.vector.memset(t[:, W + 1:W + 2], 0.0)
        sdil.append(t)

    def dilate_half(half, b_main, extra_lhs, extra_rhs, out_t, sgrp=0):
        """out = dilate3x3(binary) for one 128-row half."""
        vs = mask_ps.tile([128, W], F32, tag="mask", name="vs")
        nc.tensor.matmul(vs[:], cst["bandB"][:], b_main[:],
                         start=True, stop=(extra_lhs is None))
        if extra_lhs is not None:
            nc.tensor.matmul(vs[:], extra_lhs, extra_rhs,
                             start=False, stop=True)
        s = sdil[2 * sgrp + half]
        nc.scalar.activation(s[:, 1:W + 1], vs[:], AF.Copy)
        t = rowsm_p.tile([128, W], F32, tag="dil_t", name="dil_t")
        nc.vector.tensor_add(t[:], s[:, 0:W], s[:, 2:W + 2])
        nc.vector.tensor_add(t[:], t[:], s[:, 1:W + 1])
        nc.vector.tensor_single_scalar(out_t[:], t[:], 0.5, OP.is_gt)

    if not CONV_FP8:
        for b in range(n_img):
            for q in range(4):
                nc.gpsimd.dma_start(xsrc[b, :, q * (PX_IMG // 4):
                                    (q + 1) * (PX_IMG // 4)],
                                    xf[b, :, q * (PX_IMG // 4):
                                    (q + 1) * (PX_IMG // 4)])

    lhsA8v = cst["lhsA8"][:].rearrange("p (s m) -> p s m", s=2)
    lhsB8v = cst["lhsB8"][:].rearrange("p (s m) -> p s m", s=2)
    DR = mybir.MatmulPerfMode.DoubleRow

    pending = [None, None]

    def emit_image(b):
        # ---------------- row-layout pre-pass ----------------
        x3row, randrow, urow, bpre, prealive = [], [], [], [], []
        for half in range(2):
            xt = row_p.tile([128, W], F32, tag="x3row", name="x3row")
            nc.sync.dma_start(
                xt[:], xf[b, 3, half * HPX:(half + 1) * HPX]
                .rearrange("(p w) -> p w", w=W))
            x3row.append(xt)
            rt = row_p.tile([128, W], F32, tag="randrow", name="randrow")
            nc.sync.dma_start(
                rt[:], randf[b, half * HPX:(half + 1) * HPX]
                .rearrange("(p w) -> p w", w=W))
            randrow.append(rt)
            ut = row_p.tile([128, W], BF16, tag="urow", name="urow")
            nc.vector.tensor_single_scalar(ut[:], rt[:], EPS, OP.is_lt)
            urow.append(ut)
            bt = row_p.tile([128, W], BF16, tag="bpre", name="bpre")
            nc.vector.tensor_single_scalar(bt[:], xt[:], ALIVE_T, OP.is_gt)
            bpre.append(bt)
            prealive.append(row_p.tile([128, W], BF16, tag="prealive",
                                       name="prealive"))
        dilate_half(0, bpre[0], cst["bandClo"][:], bpre[1][:], prealive[0])
        dilate_half(1, bpre[1], cst["bandChi"][:], bpre[0][:], prealive[1])

        d3row = [d3_p.tile([128, W], BF16, tag="d3row",
                           name=f"d3row{h}") for h in range(2)]

        def load_drow(half, name):
            dr = fin_p.tile([128, C * W], BF16, tag="drow", name=name)
            for hq in range(2):
                nc.gpsimd.dma_start(
                    dr[64 * hq:64 * (hq + 1), :]
                    .rearrange("p (c w) -> p c w", w=W),
                    dscrf[b, half, 64 * hq:64 * (hq + 1)])
            return dr

        def compute_block(blk):
            r0 = blk * TR
            st3 = stacks[blk % 2]
            englist = (nc.sync, nc.sync)
            if CONV_FP8:
                if blk == 0:
                    nc.vector.memset(st3[0:16, :, 0:1, :], -0.5)
                if blk == N_BLK - 1:
                    nc.vector.memset(st3[32:48, :, TR - 1:TR, :], -0.5)
                for di in range(3):
                    rr_lo = max(0, 1 - di - r0)
                    rr_hi = min(TR, H - r0 - di + 1)
                    src = xsrc[b, :, (r0 + rr_lo + di - 1) * W:
                               (r0 + rr_hi + di - 1) * W].rearrange(
                                   "c (r w) -> c r w", w=W)
                    # A-slab: cols 1..256 <- x cols 0..255
                    englist[di % 2].dma_start(
                        st3[16 * di:16 * di + 16, 0, rr_lo:rr_hi, 1:W + 1],
                        src)
                    # B-slab: cols 0..255 <- x cols 0..255
                    englist[(di + 1) % 2].dma_start(
                        st3[16 * di:16 * di + 16, 1, rr_lo:rr_hi, 0:W], src)
            else:
                if blk == 0:
                    nc.vector.memset(st3[0:32, 0:1, :], 0.0)
                    nc.vector.memset(st3[32:64, 0:1, :], 0.0)
                if blk == N_BLK - 1:
                    nc.vector.memset(st3[32:64, TR - 1:TR, :], 0.0)
                    nc.vector.memset(st3[64:96, TR - 1:TR, :], 0.0)
                for di in range(3):
                    rr_lo = max(0, 1 - di - r0)
                    rr_hi = min(TR, H - r0 - di + 1)
                    src = xsrc[b, :, (r0 + rr_lo + di - 1) * W:
                               (r0 + rr_hi + di - 1) * W].rearrange(
                                   "c (r w) -> c r w", w=W)
                    # A-group: cols 1..257 <- x cols 0..255
                    englist[di % 2].dma_start(
                        st3[16 * di:16 * di + 16, rr_lo:rr_hi, 1:W + 1], src)
                    # B-group: cols 0..255 <- x cols 0..255 (full row)
                    englist[(di + 1) % 2].dma_start(
                        st3[48 + 16 * di:64 + 16 * di, rr_lo:rr_hi, 0:W], src)

            dgb = dgrp_p.tile([128, 512 * N_GRP], BF16, tag="d", name="d")

            def mm2_group(g, hg):
                mm = mm2_ps.tile([128, 512], F32, tag="mm2", name="mm2")
                for i in range(4):
                    nc.tensor.matmul(
                        mm[32 * i:32 * i + 32, :],
                        cst["w2dup"][:],
                        hg[:, 512 * i:512 * (i + 1)],
                        start=True, stop=True,
                        tile_position=(0, 32 * i))
                nc.scalar.activation(dgb[:, 512 * g:512 * (g + 1)], mm[:],
                                     AF.Tanh, bias=cst["b2dup"][:, 0:1])

            hg_prev = None
            for g in range(N_GRP):
                hg = hgrp_p.tile([128, 2048], BF16, tag="hgrp", name="hgrp")
                for ip in range(2):
                    acc = conv_ps.tile([128, 1024], F32, tag="conv",
                                       name="conv")
                    for i in (2 * ip, 2 * ip + 1):
                        chk = N_GRP * i + g
                        base = 512 * (i - 2 * ip)
                        if CONV_FP8:
                            for r in range(2):
                                aslr = acc[:, base + 256 * r:
                                           base + 256 * (r + 1)]
                                rr = 2 * chk + r
                                nc.tensor.matmul(
                                    aslr, lhsA8v, st3[0:48, :, rr, 0:W],
                                    start=True, stop=False, perf_mode=DR)
                                nc.tensor.matmul(
                                    aslr, lhsB8v, st3[0:48, :, rr, 1:W + 1],
                                    start=False, stop=True, perf_mode=DR)
                        else:
                            asl = acc[:, base:base + 512]
                            nc.tensor.matmul(
                                asl, cst["lhsA"][:],
                                st3[0:48, 2 * chk:2 * chk + 2, 0:W],
                                start=True, stop=False)
                            nc.tensor.matmul(
                                asl, cst["lhsB"][:],
                                st3[0:96, 2 * chk:2 * chk + 2, 1:W + 1],
                                start=False, stop=True)
                    hsl = hg[:, 1024 * ip:1024 * (ip + 1)]
                    if (blk * 8 + 2 * g + ip) % 16 < DRAIN_ACT_OF_16:
                        nc.scalar.activation(hsl, acc[:], AF.Relu,
                                             bias=cst["b1c"][:, 0:1])
                    else:
                        nc.vector.scalar_tensor_tensor(
                            hsl, acc[:], cst["b1c"][:, 0:1], zeros[:],
                            op0=OP.add, op1=OP.max)
                if hg_prev is not None:
                    mm2_group(g - 1, hg_prev)
                hg_prev = hg
            mm2_group(N_GRP - 1, hg_prev)
            # d -> DRAM scratch in drow order; image row of chunk 4i+g =
            # 32qb + 8i + 2g + r.  Per-i pieces keep SBUF dim0 = partition.
            half, qb = divmod(blk, N_BLK // 2)
            ri = 2 * N_GRP
            for i in range(4):
                nc.gpsimd.dma_start(
                    dscrf[b, half, TR * qb + ri * i:TR * qb + ri * (i + 1)]
                    .rearrange("p c w -> c p w"),
                    dgb[32 * i:32 * i + 16, :]
                    .rearrange("c (g r w) -> c (g r) w", r=2, w=W))
            # one partition-strided extract for the d3 mask rows
            nc.scalar.dma_start(
                d3row[half][TR * qb:TR * qb + TR, :],
                dgb[:].rearrange("(i p) n -> i p n", i=4)[:, 3:4, :])


        def post_binary(rows_ap_rand, rows_ap_x3, d3_ap, out_t):
            """out = (x3 + d3*(rand<eps)) > 0.1 on row-layout tiles."""
            m = rowsm_p.tile(list(out_t.shape), F32, tag="postm", name="postm")
            nc.vector.scalar_tensor_tensor(
                m[:], rows_ap_rand, EPS, d3_ap,
                op0=OP.is_lt, op1=OP.mult)
            nc.vector.tensor_add(m[:], m[:], rows_ap_x3)
            nc.vector.tensor_single_scalar(out_t[:], m[:], ALIVE_T, OP.is_gt)

        def finish_half(half, ar, dr):
            """o = clip_ch<3(ar * (xrow + d*u)); cast-stores to out."""
            xr = fin_p.tile([128, C * W], BF16, tag="xrow", name="xrow")
            for q in range(4):
                nc.gpsimd.dma_start(
                    xr[32 * q:32 * (q + 1), :]
                    .rearrange("p (c n) -> p c n", n=W),
                    xf[b, :, half * HPX + 32 * q * W:
                       half * HPX + 32 * (q + 1) * W]
                    .rearrange("c (p n) -> p c n", n=W))
            dr3 = dr[:].rearrange("p (c n) -> p c n", n=W)
            ub = urow[half][:].unsqueeze(1).broadcast_to([128, C, W])
            ab = ar[:].unsqueeze(1).broadcast_to([128, C, W])
            m = fin_p.tile([128, C * W], BF16, tag="finm", name="finm")
            m3 = m[:].rearrange("p (c n) -> p c n", n=W)
            nc.vector.tensor_mul(m3, dr3, ub)
            n_ = fin_p.tile([128, C * W], BF16, tag="finn", name="finn")
            n3 = n_[:].rearrange("p (c n) -> p c n", n=W)
            nc.vector.tensor_add(n3, m3,
                                 xr[:].rearrange("p (c n) -> p c n", n=W))
            o = fin_p.tile([128, C * W], BF16, tag="finm", name="fino")
            o3 = o[:].rearrange("p (c n) -> p c n", n=W)
            nc.vector.tensor_mul(o3, n3, ab)
            # clip channels 0..2 = contiguous cols 0..767, in place
            nc.vector.tensor_scalar(o[:, 0:3 * W], o[:, 0:3 * W], 1.0, 0.0,
                                    op0=OP.min, op1=OP.max)
            for q in range(4):
                nc.gpsimd.dma_start(
                    outf[b, :, half * HPX + 32 * q * W:
                         half * HPX + 32 * (q + 1) * W]
                    .rearrange("c (p n) -> p c n", n=W),
                    o3[32 * q:32 * (q + 1)])

        # ---------------- pipeline ----------------
        compute_block(0)
        if pending[0] is not None:
            pending[0]()
            pending[0] = None
        compute_block(1)
        dr0 = load_drow(0, "drow0")
        compute_block(2)
        compute_block(3)

        # half-0 post mask: d3 rows 0..127 + halo row 128 (= d3row1[0])
        bpost0 = rowsm_p.tile([128, W], BF16, tag="bpost0", name="bpost0")
        post_binary(randrow[0][:], x3row[0][:], d3row[0][:], bpost0)
        bp128 = rowsm_p.tile([1, W], BF16, tag="bp128", name="bp128")
        post_binary(randrow[1][0:1, :], x3row[1][0:1, :], d3row[1][0:1, :],
                    bp128)

        postal0 = rowsm_p.tile([128, W], BF16, tag="postal0", name="postal0")
        dilate_half(0, bpost0, cst["clo1"][:], bp128[:], postal0, sgrp=1)
        ar0 = rowsm_p.tile([128, W], BF16, tag="ar0", name="ar0")
        nc.vector.tensor_mul(ar0[:], prealive[0][:], postal0[:])

        finish_half(0, ar0, dr0)

        def tail1():
            # half-1 post mask + finish (deferred into next image's stream)
            dr1 = load_drow(1, "drow1")
            bpost1 = rowsm_p.tile([128, W], BF16, tag="bpost1",
                                  name="bpost1")
            post_binary(randrow[1][:], x3row[1][:], d3row[1][:], bpost1)
            postal1 = rowsm_p.tile([128, W], BF16, tag="postal1",
                                   name="postal1")
            dilate_half(1, bpost1, cst["bandChi"][:], bpost0[:], postal1,
                        sgrp=1)
            ar1 = rowsm_p.tile([128, W], BF16, tag="ar1", name="ar1")
            nc.vector.tensor_mul(ar1[:], prealive[1][:], postal1[:])
            finish_half(1, ar1, dr1)
        pending[0] = tail1

    for b in [i for _ in range(reps) for i in range(n_img)]:
        emit_image(b)
    if pending[0] is not None:
        pending[0]()

    ctx.close()


# ---------------------------------------------------------------------------

_NC_CACHE = {}


def _get_nc(n_img, reps=1):
    key = (n_img, reps)
    if key not in _NC_CACHE:
        _NC_CACHE[key] = build_program(n_img, reps)
    return _NC_CACHE[key]


def kernel(x, w1, b1, w2, b2, rand_mask):
    x = np.ascontiguousarray(np.asarray(x, np.float32))
    rand_mask = np.ascontiguousarray(np.asarray(rand_mask, np.float32))
    B = x.shape[0]
    n_img = B // N_CORES
    consts = _build_consts(w1, b1, w2, b2)
    cast = {k: np.ascontiguousarray(v.astype(mybir.dt.np(CONST_SPECS[k][1])))
            for k, v in consts.items()}

    nc = _get_nc(n_img)
    xsh8 = None
    if CONV_FP8:
        xsh8 = np.ascontiguousarray((x - 0.5).astype(mybir.dt.np(FP8)))
    in_maps = []
    for k in range(N_CORES):
        sl = slice(k * n_img, (k + 1) * n_img)
        m = dict(x=x[sl], rand=rand_mask[sl, 0], **cast)
        if CONV_FP8:
            m["xsh8"] = xsh8[sl]
        in_maps.append(m)
    res = run_bass_kernel_spmd(nc, in_maps, core_ids=list(range(N_CORES)))
    out = np.concatenate([res.results[k]["out"] for k in range(N_CORES)],
                         axis=0)
    return out.astype(np.float32)


# revision 53
# speedup vs baseline: 1.9850x; 1.0032x over previous
"""Trainium2 Bass kernel for nn_CAGetBoard (neural CA step).

Takes FULL inputs, shards batch across 8 NeuronCores (pure data parallel),
runs a Bass/Tile kernel per core, gathers the FULL output.

Per-core pipeline (B/8 images each, 4 blocks of 64 rows):
  - conv1 (Sobel folded into a 16->128 3x3 conv) runs in fp8e4m3
    DoubleRow: the host supplies xsh8 = (x - 0.5) in e4m3 (the shift
    halves quantization error; 0.5*sum(w) is folded into b1).  The
    6-copy stack is [48, 2(slab), r, 258] with A/B column alignments as
    slabs, so 2 DoubleRow matmuls (K=2x48) per 256-px row cover all 9
    taps at half cost; pads are -0.5 (the shifted-domain zero).
  - relu+bias drains PSUM->SBUF bf16, split ACT/DVE 11:5; mm2 (128->16,
    bf16, col-tiled x4 with duplicated weights) is software-pipelined
    one group behind conv; tanh drains into dgb.
  - d goes to a DRAM scratch in row-major (drow) order via per-i
    partition-strided stores; d3 mask rows are extracted straight from
    dgb.  Finishing reads d back per half-image in "ch-row" layout
    [128p=rows, 16c, 256] where u = rand<eps and the alive mask
    broadcast across the ch axis with stride-0 APs (no replication
    DMAs); all-bf16 m=d*u; n=m+x; o=n*alive; clip ch<3 = contiguous
    columns; SWDGE stores cast bf16->f32.
  - alive masks: 3x3 binary dilation via banded bf16 matmuls +
    horizontal adds on [128,256] row tiles, all kept in SBUF.
  - cross-image software pipelining: each image's half-1 masks+finish
    are deferred into the next image's block stream; DMA issue is
    spread across the SP (stack feeds), ACT (d3/dscr) and Pool
    (drow/x/out, casts) queues to avoid sequencer head-of-line
    blocking.
"""

import numpy as np

import concourse.bass as bass
import concourse.bacc as bacc
import concourse.tile as tile
import concourse.mybir as mybir
from concourse.bass_utils import run_bass_kernel_spmd

dt = mybir.dt
F32 = dt.float32
BF16 = dt.bfloat16
AF = mybir.ActivationFunctionType
OP = mybir.AluOpType

N_CORES = 8
C = 16
H = 256
W = 256
TR = 64                    # rows per compute block
WS = W + 2                 # padded row stride
N_BLK = H // TR
N_CHUNK = TR // 2          # 512-px chunks per block
N_GRP = N_CHUNK // 4       # mm2 groups per block
PX_IMG = H * W
HPX = PX_IMG // 2          # pixels per half-image
EPS = 0.5
ALIVE_T = 0.1
DRAIN_ACT_OF_16 = 9        # of every 16 relu drains, this many go to ACT
CONV_FP8 = True            # conv1 via fp8e4m3 DoubleRow on (x - 0.5)
FP8 = dt.float8e4


def _build_consts(w1, b1, w2, b2):
    w1 = np.asarray(w1, np.float32)
    w2 = np.asarray(w2, np.float32)
    sob = np.array([[-1., 0., 1.], [-2., 0., 2.], [-1., 0., 1.]], np.float32)
    W1x, W1gx, W1gy = w1[:, 0:16], w1[:, 16:32], w1[:, 32:48]
    k1f = (W1gx[:, :, None, None] * sob[None, None, :, :]
           + W1gy[:, :, None, None] * sob.T[None, None, :, :])
    k1f[:, :, 1, 1] += W1x
    lhs = np.transpose(k1f, (3, 2, 1, 0)).reshape(3, 48, 128)
    lhsA = lhs[0].copy()
    lhsB = np.concatenate([lhs[1], lhs[2]], axis=0)

    # fp8 DoubleRow pair weights: [48, 2, 128] (slab0, slab1)
    lhsA8 = np.stack([lhs[0], lhs[1]], axis=1)
    lhsB8 = np.stack([np.zeros_like(lhs[2]), lhs[2]], axis=1)
    # bias correction for the x-0.5 shift: conv(w, x) =
    # conv(w, x-0.5) + 0.5*sum_taps(w)
    ksum = k1f.sum(axis=(1, 2, 3)) * 0.5
    b1fp8 = (np.asarray(b1, np.float32) + ksum).reshape(128, 1)

    w2dup = np.zeros((128, 32), np.float32)
    w2dup[:, 0:16] = w2.T
    w2dup[:, 16:32] = w2.T

    b2dup = np.zeros((128, 1), np.float32)
    for i in range(4):
        for d in range(2):
            s = 32 * i + 16 * d
            b2dup[s:s + 16, 0] = b2

    bandB = np.zeros((128, 128), np.float32)
    for k in range(128):
        bandB[k, max(0, k - 1):k + 2] = 1.0
    bandClo = np.zeros((128, 128), np.float32)
    bandClo[0, 127] = 1.0
    bandChi = np.zeros((128, 128), np.float32)
    bandChi[127, 0] = 1.0
    clo1 = np.zeros((1, 128), np.float32)
    clo1[0, 127] = 1.0

    b1c = (b1fp8 if CONV_FP8
           else np.asarray(b1, np.float32).reshape(128, 1))
    return dict(
        lhsA=lhsA, lhsB=lhsB, lhsA8=lhsA8.reshape(48, 256),
        lhsB8=lhsB8.reshape(48, 256),
        w2dup=w2dup, b1c=b1c,
        b2dup=b2dup,
        bandB=bandB, bandClo=bandClo, bandChi=bandChi, clo1=clo1,
    )


CONST_SPECS = dict(
    lhsA=([48, 128], BF16), lhsB=([96, 128], BF16),
    lhsA8=([48, 256], FP8), lhsB8=([48, 256], FP8),
    w2dup=([128, 32], BF16), b1c=([128, 1], F32), b2dup=([128, 1], F32),
    bandB=([128, 128], BF16), bandClo=([128, 128], BF16),
    bandChi=([128, 128], BF16), clo1=([1, 128], BF16),
)


def build_program(n_img, reps=1):
    nc = bacc.Bacc("TRN2", target_bir_lowering=False)

    x_d = nc.dram_tensor("x", [n_img, C, H, W], F32, kind="ExternalInput")
    rand_d = nc.dram_tensor("rand", [n_img, H, W], F32, kind="ExternalInput")
    cst_d = {k: nc.dram_tensor(k, sh, d, kind="ExternalInput")
             for k, (sh, d) in CONST_SPECS.items()}
    out_d = nc.dram_tensor("out", [n_img, C, H, W], F32, kind="ExternalOutput")
    # d scratch in drow order: [img, half, p(=row in half), c, w]
    dscr_d = nc.dram_tensor("dscr", [n_img, 2, 128, C, W], BF16,
                            kind="Internal")

    xf = x_d.ap().rearrange("b c h w -> b c (h w)")
    outf = out_d.ap().rearrange("b c h w -> b c (h w)")
    randf = rand_d.ap().rearrange("b h w -> b (h w)")

    if CONV_FP8:
        xs_d = nc.dram_tensor("xsh8", [n_img, C, H, W], FP8,
                              kind="ExternalInput")
        xsrc = xs_d.ap().rearrange("b c h w -> b c (h w)")
    else:
        xbf_d = nc.dram_tensor("xbfscr", [n_img, C, H, W], BF16,
                               kind="Internal")
        xsrc = xbf_d.ap().rearrange("b c h w -> b c (h w)")

    with tile.TileContext(nc) as tc:
        _emit(nc, tc, n_img, xf, randf, cst_d, outf, xsrc, dscr_d.ap(), reps)
    nc.compile()
    return nc


def _emit(nc, tc, n_img, xf, randf, cst_d, outf, xsrc, dscrf, reps=1):
    from contextlib import ExitStack
    ctx = ExitStack()

    def pool(name, bufs, **kw):
        return ctx.enter_context(tc.tile_pool(name=name, bufs=bufs, **kw))

    consts = pool("consts", 1)
    stackp = pool("stack", 1)
    hgrp_p = pool("hgrp", 4)
    dgrp_p = pool("dgrp", 3)
    row_p = pool("rows", 4)
    rowsm_p = pool("rowsm", 2)
    d3_p = pool("d3", 4)
    fin_p = pool("fin", 2)
    misc_p = pool("misc", 1)
    conv_ps = pool("convps", 2, space="PSUM")
    mask_ps = pool("maskps", 1, space="PSUM")
    mm2_ps = pool("mm2ps", 2, space="PSUM")

    cst = {}
    for k, (sh, d) in CONST_SPECS.items():
        t = consts.tile(sh, d, tag=k, name=k)
        nc.scalar.dma_start(t[:], cst_d[k].ap())
        cst[k] = t

    zeros = misc_p.tile([128, 1024], F32, tag="zeros", name="zeros")
    nc.vector.memset(zeros[:], 0.0)

    # stacks.  A-layout: col k = x col k-1 (stored at cols 1..256, col 0
    # zero).  B-layout: col k = x col k (stored at cols 0..255, col 256
    # zero).  Full 256-elem rows on all DMAs.
    # bf16 path: [96, r, j] with A on partitions 0:48, B on 48:96.
    # fp8 path: [48, slab, r, j] with A = slab 0, B = slab 1 (DoubleRow).
    stacks = []
    for s in range(2):
        if CONV_FP8:
            st = stackp.tile([48, 2 * TR * WS], FP8, tag=f"stack{s}",
                             name=f"stack{s}")
            st4 = st.rearrange("p (s r j) -> p s r j", s=2, j=WS)
            nc.vector.memset(st4[0:48, 0, :, 0:1], -0.5)
            nc.vector.memset(st4[0:48, 1, :, W:W + 1], -0.5)
            stacks.append(st4)
        else:
            st = stackp.tile([96, TR * WS], BF16, tag=f"stack{s}",
                             name=f"stack{s}")
            st3 = st.rearrange("p (r j) -> p r j", j=WS)
            nc.vector.memset(st3[0:96, :, 0:1], 0.0)
            nc.vector.memset(st3[0:96, :, W:W + 1], 0.0)
            stacks.append(st3)

    sdil = []
    for s in range(4):
        t = misc_p.tile([128, WS], F32, tag=f"sdil{s}", name=f"sdil{s}")
        nc.vector.memset(t[:, 0:1], 0.0)
        nc.vector.memset(t[:, W + 1:W + 2], 0.0)
        sdil.append(t)

    def dilate_half(half, b_main, extra_lhs, extra_rhs, out_t, sgrp=0):
        """out = dilate3x3(binary) for one 128-row half."""
        vs = mask_ps.tile([128, W], F32, tag="mask", name="vs")
        nc.tensor.matmul(vs[:], cst["bandB"][:], b_main[:],
                         start=True, stop=(extra_lhs is None))
        if extra_lhs is not None:
            nc.tensor.matmul(vs[:], extra_lhs, extra_rhs,
                             start=False, stop=True)
        s = sdil[2 * sgrp + half]
        nc.scalar.activation(s[:, 1:W + 1], vs[:], AF.Copy)
        t = rowsm_p.tile([128, W], F32, tag="dil_t", name="dil_t")
        nc.vector.tensor_add(t[:], s[:, 0:W], s[:, 2:W + 2])
        nc.vector.tensor_add(t[:], t[:], s[:, 1:W + 1])
        nc.vector.tensor_single_scalar(out_t[:], t[:], 0.5, OP.is_gt)

    if not CONV_FP8:
        for b in range(n_img):
            for q in range(4):
                nc.gpsimd.dma_start(xsrc[b, :, q * (PX_IMG // 4):
                                    (q + 1) * (PX_IMG // 4)],
                                    xf[b, :, q * (PX_IMG // 4):
                                    (q + 1) * (PX_IMG // 4)])

    lhsA8v = cst["lhsA8"][:].rearrange("p (s m) -> p s m", s=2)
    lhsB8v = cst["lhsB8"][:].rearrange("p (s m) -> p s m", s=2)
    DR = mybir.MatmulPerfMode.DoubleRow

    pending = [None, None]

    def emit_image(b):
        # ---------------- row-layout pre-pass ----------------
        x3row, randrow, urow, bpre, prealive = [], [], [], [], []
        for half in range(2):
            xt = row_p.tile([128, W], F32, tag="x3row", name="x3row")
            nc.sync.dma_start(
                xt[:], xf[b, 3, half * HPX:(half + 1) * HPX]
                .rearrange("(p w) -> p w", w=W))
            x3row.append(xt)
            rt = row_p.tile([128, W], F32, tag="randrow", name="randrow")
            nc.sync.dma_start(
                rt[:], randf[b, half * HPX:(half + 1) * HPX]
                .rearrange("(p w) -> p w", w=W))
            randrow.append(rt)
            ut = row_p.tile([128, W], BF16, tag="urow", name="urow")
            nc.vector.tensor_single_scalar(ut[:], rt[:], EPS, OP.is_lt)
            urow.append(ut)
            bt = row_p.tile([128, W], BF16, tag="bpre", name="bpre")
            nc.vector.tensor_single_scalar(bt[:], xt[:], ALIVE_T, OP.is_gt)
            bpre.append(bt)
            prealive.append(row_p.tile([128, W], BF16, tag="prealive",
                                       name="prealive"))
        dilate_half(0, bpre[0], cst["bandClo"][:], bpre[1][:], prealive[0])
        dilate_half(1, bpre[1], cst["bandChi"][:], bpre[0][:], prealive[1])

        d3row = [d3_p.tile([128, W], BF16, tag="d3row",
                           name=f"d3row{h}") for h in range(2)]

        def load_drow(half, name):
            dr = fin_p.tile([128, C * W], BF16, tag="drow", name=name)
            for hq in range(2):
                nc.gpsimd.dma_start(
                    dr[64 * hq:64 * (hq + 1), :]
                    .rearrange("p (c w) -> p c w", w=W),
                    dscrf[b, half, 64 * hq:64 * (hq + 1)])
            return dr

        def compute_block(blk):
            r0 = blk * TR
            st3 = stacks[blk % 2]
            englist = (nc.sync, nc.sync)
            if CONV_FP8:
                if blk == 0:
                    nc.vector.memset(st3[0:16, :, 0:1, :], -0.5)
                if blk == N_BLK - 1:
                    nc.vector.memset(st3[32:48, :, TR - 1:TR, :], -0.5)
                for di in range(3):
                    rr_lo = max(0, 1 - di - r0)
                    rr_hi = min(TR, H - r0 - di + 1)
                    src = xsrc[b, :, (r0 + rr_lo + di - 1) * W:
                               (r0 + rr_hi + di - 1) * W].rearrange(
                                   "c (r w) -> c r w", w=W)
                    # A-slab: cols 1..256 <- x cols 0..255
                    englist[di % 2].dma_start(
                        st3[16 * di:16 * di + 16, 0, rr_lo:rr_hi, 1:W + 1],
                        src)
                    # B-slab: cols 0..255 <- x cols 0..255
                    englist[(di + 1) % 2].dma_start(
                        st3[16 * di:16 * di + 16, 1, rr_lo:rr_hi, 0:W], src)
            else:
                if blk == 0:
                    nc.vector.memset(st3[0:32, 0:1, :], 0.0)
                    nc.vector.memset(st3[32:64, 0:1, :], 0.0)
                if blk == N_BLK - 1:
                    nc.vector.memset(st3[32:64, TR - 1:TR, :], 0.0)
                    nc.vector.memset(st3[64:96, TR - 1:TR, :], 0.0)
                for di in range(3):
                    rr_lo = max(0, 1 - di - r0)
                    rr_hi = min(TR, H - r0 - di + 1)
                    src = xsrc[b, :, (r0 + rr_lo + di - 1) * W:
                               (r0 + rr_hi + di - 1) * W].rearrange(
                                   "c (r w) -> c r w", w=W)
                    # A-group: cols 1..257 <- x cols 0..255
                    englist[di % 2].dma_start(
                        st3[16 * di:16 * di + 16, rr_lo:rr_hi, 1:W + 1], src)
                    # B-group: cols 0..255 <- x cols 0..255 (full row)
                    englist[(di + 1) % 2].dma_start(
                        st3[48 + 16 * di:64 + 16 * di, rr_lo:rr_hi, 0:W], src)

            dgb = dgrp_p.tile([128, 512 * N_GRP], BF16, tag="d", name="d")

            def mm2_group(g, hg):
                mm = mm2_ps.tile([128, 512], F32, tag="mm2", name="mm2")
                for i in range(4):
                    nc.tensor.matmul(
                        mm[32 * i:32 * i + 32, :],
                        cst["w2dup"][:],
                        hg[:, 512 * i:512 * (i + 1)],
                        start=True, stop=True,
                        tile_position=(0, 32 * i))
                nc.scalar.activation(dgb[:, 512 * g:512 * (g + 1)], mm[:],
                                     AF.Tanh, bias=cst["b2dup"][:, 0:1])

            hg_prev = None
            for g in range(N_GRP):
                hg = hgrp_p.tile([128, 2048], BF16, tag="hgrp", name="hgrp")
                for ip in range(2):
                    acc = conv_ps.tile([128, 1024], F32, tag="conv",
                                       name="conv")
                    for i in (2 * ip, 2 * ip + 1):
                        chk = N_GRP * i + g
                        base = 512 * (i - 2 * ip)
                        if CONV_FP8:
                            for r in range(2):
                                aslr = acc[:, base + 256 * r:
                                           base + 256 * (r + 1)]
                                rr = 2 * chk + r
                                nc.tensor.matmul(
                                    aslr, lhsA8v, st3[0:48, :, rr, 0:W],
                                    start=True, stop=False, perf_mode=DR)
                                nc.tensor.matmul(
                                    aslr, lhsB8v, st3[0:48, :, rr, 1:W + 1],
                                    start=False, stop=True, perf_mode=DR)
                        else:
                            asl = acc[:, base:base + 512]
                            nc.tensor.matmul(
                                asl, cst["lhsA"][:],
                                st3[0:48, 2 * chk:2 * chk + 2, 0:W],
                                start=True, stop=False)
                            nc.tensor.matmul(
                                asl, cst["lhsB"][:],
                                st3[0:96, 2 * chk:2 * chk + 2, 1:W + 1],
                                start=False, stop=True)
                    hsl = hg[:, 1024 * ip:1024 * (ip + 1)]
                    if (blk * 8 + 2 * g + ip) % 16 < DRAIN_ACT_OF_16:
                        nc.scalar.activation(hsl, acc[:], AF.Relu,
                                             bias=cst["b1c"][:, 0:1])
                    else:
                        nc.vector.scalar_tensor_tensor(
                            hsl, acc[:], cst["b1c"][:, 0:1], zeros[:],
                            op0=OP.add, op1=OP.max)
                if hg_prev is not None:
                    mm2_group(g - 1, hg_prev)
                hg_prev = hg
            mm2_group(N_GRP - 1, hg_prev)
            # d -> DRAM scratch in drow order; image row of chunk 4i+g =
            # 32qb + 8i + 2g + r.  Per-i pieces keep SBUF dim0 = partition.
            half, qb = divmod(blk, N_BLK // 2)
            ri = 2 * N_GRP
            for i in range(4):
                nc.gpsimd.dma_start(
                    dscrf[b, half, TR * qb + ri * i:TR * qb + ri * (i + 1)]
                    .rearrange("p c w -> c p w"),
                    dgb[32 * i:32 * i + 16, :]
                    .rearrange("c (g r w) -> c (g r) w", r=2, w=W))
            # one partition-strided extract for the d3 mask rows
            nc.scalar.dma_start(
                d3row[half][TR * qb:TR * qb + TR, :],
                dgb[:].rearrange("(i p) n -> i p n", i=4)[:, 3:4, :])


        def post_binary(rows_ap_rand, rows_ap_x3, d3_ap, out_t):
            """out = (x3 + d3*(rand<eps)) > 0.1 on row-layout tiles."""
            m = rowsm_p.tile(list(out_t.shape), F32, tag="postm", name="postm")
            nc.vector.scalar_tensor_tensor(
                m[:], rows_ap_rand, EPS, d3_ap,
                op0=OP.is_lt, op1=OP.mult)
            nc.vector.tensor_add(m[:], m[:], rows_ap_x3)
            nc.vector.tensor_single_scalar(out_t[:], m[:], ALIVE_T, OP.is_gt)

        def finish_half(half, ar, dr):
            """o = clip_ch<3(ar * (xrow + d*u)); cast-stores to out."""
            xr = fin_p.tile([128, C * W], BF16, tag="xrow", name="xrow")
            for q in range(4):
                nc.gpsimd.dma_start(
                    xr[32 * q:32 * (q + 1), :]
                    .rearrange("p (c n) -> p c n", n=W),
                    xf[b, :, half * HPX + 32 * q * W:
                       half * HPX + 32 * (q + 1) * W]
                    .rearrange("c (p n) -> p c n", n=W))
            dr3 = dr[:].rearrange("p (c n) -> p c n", n=W)
            ub = urow[half][:].unsqueeze(1).broadcast_to([128, C, W])
            ab = ar[:].unsqueeze(1).broadcast_to([128, C, W])
            m = fin_p.tile([128, C * W], BF16, tag="finm", name="finm")
            m3 = m[:].rearrange("p (c n) -> p c n", n=W)
            nc.vector.tensor_mul(m3, dr3, ub)
            n_ = fin_p.tile([128, C * W], BF16, tag="finn", name="finn")
            n3 = n_[:].rearrange("p (c n) -> p c n", n=W)
            nc.vector.tensor_add(n3, m3,
                                 xr[:].rearrange("p (c n) -> p c n", n=W))
            o = fin_p.tile([128, C * W], BF16, tag="finm", name="fino")
            o3 = o[:].rearrange("p (c n) -> p c n", n=W)
            nc.vector.tensor_mul(o3, n3, ab)
            # clip channels 0..2 = contiguous cols 0..767, in place
            nc.vector.tensor_scalar(o[:, 0:3 * W], o[:, 0:3 * W], 1.0, 0.0,
                                    op0=OP.min, op1=OP.max)
            for q in range(4):
                nc.gpsimd.dma_start(
                    outf[b, :, half * HPX + 32 * q * W:
                         half * HPX + 32 * (q + 1) * W]
                    .rearrange("c (p n) -> p c n", n=W),
                    o3[32 * q:32 * (q + 1)])

        # ---------------- pipeline ----------------
        compute_block(0)
        if pending[0] is not None:
            pending[0]()
            pending[0] = None
        compute_block(1)
        dr0 = load_drow(0, "drow0")
        compute_block(2)
        compute_block(3)

        # half-0 post mask: d3 rows 0..127 + halo row 128 (= d3row1[0])
        bpost0 = rowsm_p.tile([128, W], BF16, tag="bpost0", name="bpost0")
        post_binary(randrow[0][:], x3row[0][:], d3row[0][:], bpost0)
        bp128 = rowsm_p.tile([1, W], BF16, tag="bp128", name="bp128")
        post_binary(randrow[1][0:1, :], x3row[1][0:1, :], d3row[1][0:1, :],
                    bp128)

        postal0 = rowsm_p.tile([128, W], BF16, tag="postal0", name="postal0")
        dilate_half(0, bpost0, cst["clo1"][:], bp128[:], postal0, sgrp=1)
        ar0 = rowsm_p.tile([128, W], BF16, tag="ar0", name="ar0")
        nc.vector.tensor_mul(ar0[:], prealive[0][:], postal0[:])

        finish_half(0, ar0, dr0)

        def tail1():
            # half-1 post mask + finish (deferred into next image's stream)
            dr1 = load_drow(1, "drow1")
            bpost1 = rowsm_p.tile([128, W], BF16, tag="bpost1",
                                  name="bpost1")
            post_binary(randrow[1][:], x3row[1][:], d3row[1][:], bpost1)
            postal1 = rowsm_p.tile([128, W], BF16, tag="postal1",
                                   name="postal1")
            dilate_half(1, bpost1, cst["bandChi"][:], bpost0[:], postal1,
                        sgrp=1)
            ar1 = rowsm_p.tile([128, W], BF16, tag="ar1", name="ar1")
            nc.vector.tensor_mul(ar1[:], prealive[1][:], postal1[:])
            finish_half(1, ar1, dr1)
        pending[0] = tail1

    for b in [i for _ in range(reps) for i in range(n_img)]:
        emit_image(b)
    if pending[0] is not None:
        pending[0]()

    ctx.close()


# ---------------------------------------------------------------------------

_NC_CACHE = {}


def _get_nc(n_img, reps=1):
    key = (n_img, reps)
    if key not in _NC_CACHE:
        _NC_CACHE[key] = build_program(n_img, reps)
    return _NC_CACHE[key]


def kernel(x, w1, b1, w2, b2, rand_mask):
    x = np.ascontiguousarray(np.asarray(x, np.float32))
    rand_mask = np.ascontiguousarray(np.asarray(rand_mask, np.float32))
    B = x.shape[0]
    n_img = B // N_CORES
    consts = _build_consts(w1, b1, w2, b2)
    cast = {k: np.ascontiguousarray(v.astype(mybir.dt.np(CONST_SPECS[k][1])))
            for k, v in consts.items()}

    nc = _get_nc(n_img)
    xsh8 = None
    if CONV_FP8:
        xsh8 = np.ascontiguousarray((x - 0.5).astype(mybir.dt.np(FP8)))
    in_maps = []
    for k in range(N_CORES):
        sl = slice(k * n_img, (k + 1) * n_img)
        m = dict(x=x[sl], rand=rand_mask[sl, 0], **cast)
        if CONV_FP8:
            m["xsh8"] = xsh8[sl]
        in_maps.append(m)
    res = run_bass_kernel_spmd(nc, in_maps, core_ids=list(range(N_CORES)))
    out = np.concatenate([res.results[k]["out"] for k in range(N_CORES)],
                         axis=0)
    return out.astype(np.float32)
